# revision 12
# baseline (speedup 1.0000x reference)
"""Bayesian linear layer (mean-field reparameterization) on 8 TRN2 NeuronCores.

out[b,o] = sum_i (eps_w[b,o,i]*exp(w_psi[o,i]) + w_mu[o,i]) * x[b,i]
         + eps_b[b,o]*exp(b_psi[o]) + b_mu[o]

v6 strategy (data-parallel over batch, 32 batches/core):
 - eps_w shipped as fp8 e4m3 [i, b, o] (~2.7% rms elem noise on a term
   that is ~24% of the output -> ~6e-3 total rel err, gate is 2e-2).
 - The s=exp(psi) elementwise multiply must run on DVE (TT bf16 2x mode,
   4.42us per [128,8,1024] tile, measured); GPSIMD TT poisons DVE via
   the shared SBUF port (4.3x slowdown, measured) so all 32 tiles
   multiply on DVE: ~141us/rep, the kernel's wall.
 - To keep the DMA fabric under that wall, only 20/32 tiles use the
   SWDGE cast-DMA (fp8->bf16 doubles SBUF-side bytes); 12/32 are DMA'd
   raw fp8 (HWDGE) and upconverted on the idle scalar engine
   (7.1us/tile measured). DMA ~140us, ACT ~100us, both under DVE.
 - PE: per-batch matvecs col-tiled 4-wide (tile_position (0,32j),
   j=b%4): 4 M=32 matmuls stream concurrently (~2.6x measured).
   mu-term matmuls (x @ muT, bf16) and the b_mu row (K=1 ones matmul)
   accumulate into PSUM region 0. Epilogue: one ACT copy PSUM->SBUF,
   then region sums + eps_b*exp(b_psi) bias on DVE in SBUF.
 - params psi/mu/x/b_mu shipped bf16 (mu-term keeps bf16 accuracy).
"""

import numpy as np
import ml_dtypes

import os

BS, OUT, IN = 256, 1024, 1024
NCORES = 8
BPC = BS // NCORES          # 32 batches per core
ICH = IN // 128             # 8 i-chunks
EBUFS_DEFAULT = int(os.environ.get("BK_EBUFS", "3"))
E8BUFS_DEFAULT = int(os.environ.get("BK_E8BUFS", "3"))
ACT_PAIRS = int(os.environ.get("BK_NACTP", "7"))  # raw+ACT-upconvert ic-pair tiles (of 16)
MU_CPT = 4                  # i-chunks per mu/psi DMA

_cache = {}


def _build(reps, loop=False, ebufs=None, pbufs=None):
    EBUFS = ebufs or EBUFS_DEFAULT
    import concourse.bass as bass
    import concourse.mybir as mybir
    import concourse.bacc as bacc
    from concourse import tile

    f32 = mybir.dt.float32
    bf16 = mybir.dt.bfloat16
    fp8 = mybir.dt.float8e4
    mult = mybir.AluOpType.mult
    add = mybir.AluOpType.add

    nc = bacc.Bacc(None, target_bir_lowering=False)

    d_eps = nc.dram_tensor("epsT", [IN, BPC, OUT], fp8, kind="ExternalInput")
    d_xT = nc.dram_tensor("xT", [IN, BPC], bf16, kind="ExternalInput")
    d_psiT = nc.dram_tensor("psiT", [IN, OUT], bf16, kind="ExternalInput")
    d_muT = nc.dram_tensor("muT", [IN, OUT], bf16, kind="ExternalInput")
    d_eb = nc.dram_tensor("eps_b", [BPC, OUT], f32, kind="ExternalInput")
    d_bpsi = nc.dram_tensor("bpsi", [1, OUT], f32, kind="ExternalInput")
    d_bmu = nc.dram_tensor("bmu16", [1, OUT], bf16, kind="ExternalInput")
    if loop:
        d_it = nc.dram_tensor("iters", [1, 1], mybir.dt.int32,
                              kind="ExternalInput")
    d_out = nc.dram_tensor("out", [BPC, OUT], f32, kind="ExternalOutput")

    with tile.TileContext(nc) as tc:
        with tc.tile_pool(name="const", bufs=1) as cpool, \
             tc.tile_pool(name="eps", bufs=EBUFS) as epool, \
             tc.tile_pool(name="eps8", bufs=E8BUFS_DEFAULT) as e8pool, \
             tc.tile_pool(name="ps", bufs=2, space="PSUM") as pspool:

            sTbA = cpool.tile([128, ICH // 2, OUT], bf16, name="sTbA")
            sTbB = cpool.tile([128, ICH // 2, OUT], bf16, name="sTbB")
            xTb = cpool.tile([128, ICH, BPC], bf16, name="xTb")
            xdiag = cpool.tile([128, ICH, BPC, BPC], bf16, name="xdiag")
            ones = cpool.tile([1, BPC], bf16, name="ones")
            ebt = cpool.tile([BPC, OUT], f32, name="ebt")
            sbrow = cpool.tile([1, OUT], f32, name="sbrow")
            sb_bc = cpool.tile([BPC, OUT], f32, name="sb_bc")
            bmurow = cpool.tile([1, OUT], bf16, name="bmurow")
            t1 = cpool.tile([BPC, OUT], f32, name="t1")
            t2 = cpool.tile([BPC, OUT], f32, name="t2")
            ebs = cpool.tile([BPC, OUT], f32, name="ebs")
            out_sb = cpool.tile([BPC, OUT], f32, name="out_sb")

            nc.vector.memset(xdiag[:], 0.0)
            nc.vector.memset(ones[:], 1.0)

            def emit(rep):
                # ---- prologue: params, exp(psi) -> bf16 ----
                for t in range(ICH // MU_CPT):
                    pt = epool.tile([128, MU_CPT, OUT], bf16,
                                    name=f"psi_{rep}_{t}", tag="eps")
                    nc.scalar.dma_start(
                        out=pt[:],
                        in_=d_psiT[t * MU_CPT * 128:(t + 1) * MU_CPT * 128, :]
                        .rearrange("(s p) o -> p s o", p=128))
                    nc.scalar.activation(
                        (sTbA if t == 0 else sTbB)[:], pt[:],
                        mybir.ActivationFunctionType.Exp)
                nc.scalar.dma_start(out=xTb[:], in_=d_xT[:]
                                    .rearrange("(c p) b -> p c b", p=128))
                # xdiag[:, :, b, b] = x[b, :] ; off-diagonal stays zero
                for b in range(BPC):
                    nc.scalar.copy(xdiag[:, :, b, b], xTb[:, :, b])

                nc.sync.dma_start(out=ebt[:], in_=d_eb[:])
                nc.sync.dma_start(out=sbrow[:], in_=d_bpsi[:])
                nc.scalar.activation(sbrow[:], sbrow[:],
                                     mybir.ActivationFunctionType.Exp)
                nc.gpsimd.partition_broadcast(sb_bc[:], sbrow[:])
                nc.sync.dma_start(out=bmurow[:], in_=d_bmu[:])

                # PSUM: 4 col-group regions of 32 rows; region j
                # accumulates batches with b%4==j (plus mu-term+b_mu in 0)
                accps = pspool.tile([128, OUT], f32, name=f"acc_{rep}",
                                    tag="ps")

                # mu-term into region 0: acc[b,o] += sum_i x[b,i]*mu[o,i]
                mt = epool.tile([128, ICH, OUT], bf16,
                                name=f"mu_{rep}", tag="eps")
                nc.scalar.dma_start(
                    out=mt[:],
                    in_=d_muT[:].rearrange("(s p) o -> p s o", p=128))
                for ic in range(ICH):
                    for h in range(2):
                        nc.tensor.matmul(
                            accps[0:BPC, h * 512:(h + 1) * 512],
                            xTb[:, ic, :],
                            mt[:, ic, h * 512:(h + 1) * 512],
                            start=(ic == 0), stop=False)
                # b_mu row broadcast into all 32 rows of region 0 (K=1)
                for h in range(2):
                    nc.tensor.matmul(
                        accps[0:BPC, h * 512:(h + 1) * 512],
                        ones[:], bmurow[0:1, h * 512:(h + 1) * 512],
                        start=False, stop=False)

                # ---- main loop: eps-term matvecs, col-tiled 4-wide ----
                # tiles carry an ic-PAIR [128, 2, 8, 1024] to halve DMA
                # count (per-DMA completion latency was the binding cost)
                BG = 8                       # batches per eps tile
                NG = BPC // BG
                NP = ICH // 2                # 4 ic-pairs
                started = [True, False, False, False]

                def is_act(p, g):
                    return (g == 1 and p < ACT_PAIRS) or \
                           (g == 2 and p < ACT_PAIRS - 4)
                for p in range(NP):
                    for g in range(NG):
                        e = epool.tile([128, 2, BG, OUT], bf16,
                                       name=f"e_{rep}_{p}_{g}",
                                       tag="eps")
                        src_ap = d_eps[2 * p * 128:(2 * p + 2) * 128,
                                       g * BG:(g + 1) * BG, :] \
                            .rearrange("(c q) b o -> q c b o", q=128)
                        if is_act(p, g):
                            # raw fp8 DMA (HWDGE) + scalar-engine upconvert
                            e8 = e8pool.tile([128, 2, BG, OUT], fp8,
                                             name=f"e8_{rep}_{p}_{g}",
                                             tag="e8")
                            nc.sync.dma_start(out=e8[:], in_=src_ap)
                            nc.scalar.copy(e[:], e8[:])
                        else:
                            # cast-DMA fp8->bf16 (SWDGE)
                            nc.gpsimd.dma_start(out=e[:], in_=src_ap)
                        # DVE multiply by s, in place
                        sT = sTbA if p < NP // 2 else sTbB
                        ps = (2 * p) % (ICH // 2)
                        nc.vector.tensor_tensor(
                            e[:], e[:],
                            sT[:, ps:ps + 2, :]
                            .rearrange("q c (u o) -> q c u o", u=1)
                            .broadcast_to((128, 2, BG, OUT)), mult)
                        last = p == NP - 1
                        for c in range(2):
                            ic = 2 * p + c
                            for bj in range(BG):
                                b = g * BG + bj
                                j = b % 4
                                st = not started[j]
                                started[j] = True
                                sp = (last and g == NG - 1 and c == 1
                                      and bj == j + 4)
                                for h in range(2):
                                    nc.tensor.matmul(
                                        accps[32 * j:32 * (j + 1),
                                              h * 512:(h + 1) * 512],
                                        xdiag[:, ic, b, :],
                                        e[:, c, bj,
                                          h * 512:(h + 1) * 512],
                                        start=st, stop=sp,
                                        tile_position=(0, 32 * j))

                # ---- epilogue: bias + region sums (1 PSUM input/op;
                # PSUM base may differ from the SBUF operand's base) ----
                nc.vector.tensor_tensor(ebs[:], ebt[:], sb_bc[:], mult)
                nc.vector.tensor_tensor(t1[:], accps[0:32, :], ebs[:], add)
                nc.vector.tensor_tensor(t2[:], accps[32:64, :], t1[:], add)
                nc.vector.tensor_tensor(t1[:], accps[64:96, :], t2[:], add)
                nc.vector.tensor_tensor(out_sb[:], accps[96:128, :], t1[:],
                                        add)
                nc.sync.dma_start(out=d_out[:], in_=out_sb[:])

            if loop:
                it_sb = cpool.tile([1, 1], mybir.dt.int32, name="it_sb")
                nc.sync.dma_start(out=it_sb[:], in_=d_it[:])
                regs = []
                for et in mybir.ALL_ENGINES:
                    eng = nc.engines[et]
                    r = eng.alloc_register(f"iters_{et.name}")
                    eng.reg_load(r, it_sb[0:1, 0:1])
                    regs.append(r)
                iters_val = bass.make_scalar_value(
                    bass.RegisterHandles(regs), min_val=1, max_val=1 << 20)
                with tc.For_i(0, iters_val, 1,
                              hint_engines=(mybir.EngineType.PE,
                                            mybir.EngineType.DVE,
                                            mybir.EngineType.SP)):
                    emit(0)
            else:
                for rep in range(reps):
                    emit(rep)

    nc.compile()
    return nc


def _get_nc(reps, loop=False, ebufs=None, pbufs=None):
    key = (reps, loop, ebufs, pbufs)
    if key not in _cache:
        _cache[key] = _build(reps, loop, ebufs, pbufs)
    return _cache[key]


def _prepare_inmaps(x, weight_mu, weight_psi, bias_mu, bias_psi, eps_w, eps_b):
    x = np.asarray(x, dtype=np.float32)
    weight_mu = np.asarray(weight_mu, dtype=np.float32)
    weight_psi = np.asarray(weight_psi, dtype=np.float32)
    bias_mu = np.asarray(bias_mu, dtype=np.float32)
    bias_psi = np.asarray(bias_psi, dtype=np.float32)
    eps_w = np.asarray(eps_w, dtype=np.float32)
    eps_b = np.asarray(eps_b, dtype=np.float32)

    # fp8 cast once on the full tensor (cheaper transposes afterwards).
    # TRN FP8_EXP4 tops out at +-240; N(0,1) data never gets near it.
    eps8 = eps_w.astype(ml_dtypes.float8_e4m3)
    psiT = np.ascontiguousarray(weight_psi.T).astype(ml_dtypes.bfloat16)
    muT = np.ascontiguousarray(weight_mu.T).astype(ml_dtypes.bfloat16)
    bpsi = bias_psi.reshape(1, OUT)
    bmu16 = bias_mu.reshape(1, OUT).astype(ml_dtypes.bfloat16)

    in_maps = []
    for c in range(NCORES):
        sl = slice(c * BPC, (c + 1) * BPC)
        in_maps.append({
            "epsT": np.ascontiguousarray(eps8[sl].transpose(2, 0, 1)),
            "xT": np.ascontiguousarray(x[sl].T).astype(ml_dtypes.bfloat16),
            "psiT": psiT,
            "muT": muT,
            "eps_b": np.ascontiguousarray(eps_b[sl]),
            "bpsi": bpsi,
            "bmu16": bmu16,
        })
    return in_maps


def _run(in_maps, reps=1, loop_iters=None, ebufs=None, pbufs=None, **kw):
    from concourse.bass_utils import run_bass_kernel_spmd
    nc = _get_nc(reps, loop=loop_iters is not None, ebufs=ebufs, pbufs=pbufs)
    if loop_iters is not None:
        it = np.array([[loop_iters]], dtype=np.int32)
        in_maps = [{**m, "iters": it} for m in in_maps]
    res = run_bass_kernel_spmd(nc, in_maps, core_ids=list(range(NCORES)))
    return np.concatenate([res.results[c]["out"] for c in range(NCORES)],
                          axis=0)


def kernel(x, weight_mu, weight_psi, bias_mu, bias_psi, eps_w, eps_b):
    in_maps = _prepare_inmaps(x, weight_mu, weight_psi, bias_mu, bias_psi,
                              eps_w, eps_b)
    try:
        return _run(in_maps)
    except Exception:
        _cache.clear()
        return _run(in_maps)


# revision 14
# speedup vs baseline: 1.1324x; 1.1324x over previous
"""Bayesian linear layer (mean-field reparameterization) on 8 TRN2 NeuronCores.

out[b,o] = sum_i (eps_w[b,o,i]*exp(w_psi[o,i]) + w_mu[o,i]) * x[b,i]
         + eps_b[b,o]*exp(b_psi[o]) + b_mu[o]

v6 strategy (data-parallel over batch, 32 batches/core):
 - eps_w shipped as fp8 e4m3 [i, b, o] (~2.7% rms elem noise on a term
   that is ~24% of the output -> ~6e-3 total rel err, gate is 2e-2).
 - The s=exp(psi) elementwise multiply must run on DVE (TT bf16 2x mode,
   4.42us per [128,8,1024] tile, measured); GPSIMD TT poisons DVE via
   the shared SBUF port (4.3x slowdown, measured) so all 32 tiles
   multiply on DVE: ~141us/rep, the kernel's wall.
 - To keep the DMA fabric under that wall, only 20/32 tiles use the
   SWDGE cast-DMA (fp8->bf16 doubles SBUF-side bytes); 12/32 are DMA'd
   raw fp8 (HWDGE) and upconverted on the idle scalar engine
   (7.1us/tile measured). DMA ~140us, ACT ~100us, both under DVE.
 - PE: per-batch matvecs col-tiled 4-wide (tile_position (0,32j),
   j=b%4): 4 M=32 matmuls stream concurrently (~2.6x measured).
   mu-term matmuls (x @ muT, bf16) and the b_mu row (K=1 ones matmul)
   accumulate into PSUM region 0. Epilogue: one ACT copy PSUM->SBUF,
   then region sums + eps_b*exp(b_psi) bias on DVE in SBUF.
 - params psi/mu/x/b_mu shipped bf16 (mu-term keeps bf16 accuracy).
"""

import numpy as np
import ml_dtypes

import os

BS, OUT, IN = 256, 1024, 1024
NCORES = 8
BPC = BS // NCORES          # 32 batches per core
ICH = IN // 128             # 8 i-chunks
EBUFS_DEFAULT = int(os.environ.get("BK_EBUFS", "7"))
E8BUFS_DEFAULT = int(os.environ.get("BK_E8BUFS", "3"))
ACT_TILES = int(os.environ.get("BK_NACT", "14"))  # raw+ACT-upconvert tiles
MU_CPT = 4                  # i-chunks per mu/psi DMA

_cache = {}


def _build(reps, loop=False, ebufs=None, pbufs=None):
    EBUFS = ebufs or EBUFS_DEFAULT
    import concourse.bass as bass
    import concourse.mybir as mybir
    import concourse.bacc as bacc
    from concourse import tile

    f32 = mybir.dt.float32
    bf16 = mybir.dt.bfloat16
    fp8 = mybir.dt.float8e4
    mult = mybir.AluOpType.mult
    add = mybir.AluOpType.add

    nc = bacc.Bacc(None, target_bir_lowering=False)

    d_eps = nc.dram_tensor("epsT", [IN, BPC, OUT], fp8, kind="ExternalInput")
    d_xT = nc.dram_tensor("xT", [IN, BPC], bf16, kind="ExternalInput")
    d_psiT = nc.dram_tensor("psiT", [IN, OUT], bf16, kind="ExternalInput")
    d_muT = nc.dram_tensor("muT", [IN, OUT], bf16, kind="ExternalInput")
    d_eb = nc.dram_tensor("eps_b", [BPC, OUT], f32, kind="ExternalInput")
    d_bpsi = nc.dram_tensor("bpsi", [1, OUT], f32, kind="ExternalInput")
    d_bmu = nc.dram_tensor("bmu16", [1, OUT], bf16, kind="ExternalInput")
    if loop:
        d_it = nc.dram_tensor("iters", [1, 1], mybir.dt.int32,
                              kind="ExternalInput")
    d_out = nc.dram_tensor("out", [BPC, OUT], f32, kind="ExternalOutput")

    with tile.TileContext(nc) as tc:
        with tc.tile_pool(name="const", bufs=1) as cpool, \
             tc.tile_pool(name="eps", bufs=EBUFS) as epool, \
             tc.tile_pool(name="eps8", bufs=E8BUFS_DEFAULT) as e8pool, \
             tc.tile_pool(name="ps", bufs=4, space="PSUM") as pspool:

            sTbA = cpool.tile([128, ICH // 2, OUT], bf16, name="sTbA")
            sTbB = cpool.tile([128, ICH // 2, OUT], bf16, name="sTbB")
            xTb = cpool.tile([128, ICH, BPC], bf16, name="xTb")
            xdiag = cpool.tile([128, ICH, BPC, BPC], bf16, name="xdiag")
            ones = cpool.tile([1, BPC], bf16, name="ones")
            ebt = cpool.tile([BPC, OUT], f32, name="ebt")
            sbrow = cpool.tile([1, OUT], f32, name="sbrow")
            sb_bc = cpool.tile([BPC, OUT], f32, name="sb_bc")
            bmurow = cpool.tile([1, OUT], bf16, name="bmurow")
            t1 = cpool.tile([BPC, OUT], f32, name="t1")
            t2 = cpool.tile([BPC, OUT], f32, name="t2")
            ebs = cpool.tile([BPC, OUT], f32, name="ebs")
            out_sb = cpool.tile([BPC, OUT], f32, name="out_sb")

            nc.vector.memset(xdiag[:], 0.0)
            nc.vector.memset(ones[:], 1.0)

            def emit(rep):
                # ---- prologue: params, exp(psi) -> bf16 ----
                for t in range(ICH // MU_CPT):
                    pt = epool.tile([128, MU_CPT, OUT], bf16,
                                    name=f"psi_{rep}_{t}", tag="eps")
                    nc.scalar.dma_start(
                        out=pt[:],
                        in_=d_psiT[t * MU_CPT * 128:(t + 1) * MU_CPT * 128, :]
                        .rearrange("(s p) o -> p s o", p=128))
                    nc.scalar.activation(
                        (sTbA if t == 0 else sTbB)[:], pt[:],
                        mybir.ActivationFunctionType.Exp)
                nc.scalar.dma_start(out=xTb[:], in_=d_xT[:]
                                    .rearrange("(c p) b -> p c b", p=128))
                # xdiag[:, :, b, b] = x[b, :] ; off-diagonal stays zero
                for b in range(BPC):
                    nc.scalar.copy(xdiag[:, :, b, b], xTb[:, :, b])

                nc.sync.dma_start(out=ebt[:], in_=d_eb[:])
                nc.sync.dma_start(out=sbrow[:], in_=d_bpsi[:])
                nc.scalar.activation(sbrow[:], sbrow[:],
                                     mybir.ActivationFunctionType.Exp)
                nc.gpsimd.partition_broadcast(sb_bc[:], sbrow[:])
                nc.sync.dma_start(out=bmurow[:], in_=d_bmu[:])

                # PSUM: 4 col-group regions of 32 rows; region j
                # accumulates batches with b%4==j (plus mu-term+b_mu in 0)
                accps = pspool.tile([128, OUT], f32, name=f"acc_{rep}",
                                    tag="ps")

                # mu-term into region 0: acc[b,o] += sum_i x[b,i]*mu[o,i]
                for t in range(ICH // MU_CPT):
                    mt = epool.tile([128, MU_CPT, OUT], bf16,
                                    name=f"mu_{rep}_{t}", tag="eps")
                    nc.scalar.dma_start(
                        out=mt[:],
                        in_=d_muT[t * MU_CPT * 128:(t + 1) * MU_CPT * 128, :]
                        .rearrange("(s p) o -> p s o", p=128))
                    for s in range(MU_CPT):
                        ic = t * MU_CPT + s
                        for h in range(2):
                            nc.tensor.matmul(
                                accps[0:BPC, h * 512:(h + 1) * 512],
                                xTb[:, ic, :],
                                mt[:, s, h * 512:(h + 1) * 512],
                                start=(ic == 0), stop=False)
                # b_mu row broadcast into all 32 rows of region 0 (K=1)
                for h in range(2):
                    nc.tensor.matmul(
                        accps[0:BPC, h * 512:(h + 1) * 512],
                        ones[:], bmurow[0:1, h * 512:(h + 1) * 512],
                        start=False, stop=False)

                # ---- main loop: eps-term matvecs, col-tiled 4-wide ----
                BG = 8                       # batches per eps tile
                NG = BPC // BG
                started = [True, False, False, False]
                # raw+ACT-upconvert assignment: g==1 all ics, g==2 even ics
                def is_act(ic, g):
                    k = ic * NG + g
                    return (g == 1 and ic < ACT_TILES) or \
                           (g == 2 and ic < ACT_TILES - 8)
                for ic in range(ICH):
                    for g in range(NG):
                        e = epool.tile([128, BG, OUT], bf16,
                                       name=f"e_{rep}_{ic}_{g}",
                                       tag="eps")
                        if is_act(ic, g):
                            # raw fp8 DMA (HWDGE) + scalar-engine upconvert
                            e8 = e8pool.tile([128, BG, OUT], fp8,
                                             name=f"e8_{rep}_{ic}_{g}",
                                             tag="e8")
                            nc.sync.dma_start(
                                out=e8[:],
                                in_=d_eps[ic * 128:(ic + 1) * 128,
                                          g * BG:(g + 1) * BG, :])
                            nc.scalar.copy(e[:], e8[:])
                        else:
                            # cast-DMA fp8->bf16 (SWDGE)
                            nc.gpsimd.dma_start(
                                out=e[:],
                                in_=d_eps[ic * 128:(ic + 1) * 128,
                                          g * BG:(g + 1) * BG, :])
                        # DVE multiply by s, in place
                        sT = sTbA if ic < ICH // 2 else sTbB
                        ics = ic % (ICH // 2)
                        nc.vector.tensor_tensor(
                            e[:], e[:],
                            sT[:, ics:ics + 1, :].broadcast_to(
                                (128, BG, OUT)), mult)
                        p2 = e
                        last = ic == ICH - 1
                        for bj in range(BG):
                            b = g * BG + bj
                            j = b % 4
                            st = not started[j]
                            started[j] = True
                            # last matmul for region j: ic==7, last bj in
                            # this tile with bj%4==j (bj = j+4), last g
                            sp = last and g == NG - 1 and bj == j + 4
                            for h in range(2):
                                nc.tensor.matmul(
                                    accps[32 * j:32 * (j + 1),
                                          h * 512:(h + 1) * 512],
                                    xdiag[:, ic, b, :],
                                    p2[:, bj, h * 512:(h + 1) * 512],
                                    start=st, stop=sp,
                                    tile_position=(0, 32 * j))

                # ---- epilogue: bias + region sums (1 PSUM input/op;
                # PSUM base may differ from the SBUF operand's base) ----
                nc.vector.tensor_tensor(ebs[:], ebt[:], sb_bc[:], mult)
                nc.vector.tensor_tensor(t1[:], accps[0:32, :], ebs[:], add)
                nc.vector.tensor_tensor(t2[:], accps[32:64, :], t1[:], add)
                nc.vector.tensor_tensor(t1[:], accps[64:96, :], t2[:], add)
                nc.vector.tensor_tensor(out_sb[:], accps[96:128, :], t1[:],
                                        add)
                nc.sync.dma_start(out=d_out[:], in_=out_sb[:])

            if loop:
                it_sb = cpool.tile([1, 1], mybir.dt.int32, name="it_sb")
                nc.sync.dma_start(out=it_sb[:], in_=d_it[:])
                regs = []
                for et in mybir.ALL_ENGINES:
                    eng = nc.engines[et]
                    r = eng.alloc_register(f"iters_{et.name}")
                    eng.reg_load(r, it_sb[0:1, 0:1])
                    regs.append(r)
                iters_val = bass.make_scalar_value(
                    bass.RegisterHandles(regs), min_val=1, max_val=1 << 20)
                with tc.For_i(0, iters_val, 1,
                              hint_engines=(mybir.EngineType.PE,
                                            mybir.EngineType.DVE,
                                            mybir.EngineType.SP)):
                    emit(0)
            else:
                for rep in range(reps):
                    emit(rep)

    nc.compile()
    return nc


def _get_nc(reps, loop=False, ebufs=None, pbufs=None):
    key = (reps, loop, ebufs, pbufs)
    if key not in _cache:
        _cache[key] = _build(reps, loop, ebufs, pbufs)
    return _cache[key]


def _prepare_inmaps(x, weight_mu, weight_psi, bias_mu, bias_psi, eps_w, eps_b):
    x = np.asarray(x, dtype=np.float32)
    weight_mu = np.asarray(weight_mu, dtype=np.float32)
    weight_psi = np.asarray(weight_psi, dtype=np.float32)
    bias_mu = np.asarray(bias_mu, dtype=np.float32)
    bias_psi = np.asarray(bias_psi, dtype=np.float32)
    eps_w = np.asarray(eps_w, dtype=np.float32)
    eps_b = np.asarray(eps_b, dtype=np.float32)

    # fp8 cast once on the full tensor (cheaper transposes afterwards).
    # TRN FP8_EXP4 tops out at +-240; N(0,1) data never gets near it.
    eps8 = eps_w.astype(ml_dtypes.float8_e4m3)
    psiT = np.ascontiguousarray(weight_psi.T).astype(ml_dtypes.bfloat16)
    muT = np.ascontiguousarray(weight_mu.T).astype(ml_dtypes.bfloat16)
    bpsi = bias_psi.reshape(1, OUT)
    bmu16 = bias_mu.reshape(1, OUT).astype(ml_dtypes.bfloat16)

    in_maps = []
    for c in range(NCORES):
        sl = slice(c * BPC, (c + 1) * BPC)
        in_maps.append({
            "epsT": np.ascontiguousarray(eps8[sl].transpose(2, 0, 1)),
            "xT": np.ascontiguousarray(x[sl].T).astype(ml_dtypes.bfloat16),
            "psiT": psiT,
            "muT": muT,
            "eps_b": np.ascontiguousarray(eps_b[sl]),
            "bpsi": bpsi,
            "bmu16": bmu16,
        })
    return in_maps


def _run(in_maps, reps=1, loop_iters=None, ebufs=None, pbufs=None, **kw):
    from concourse.bass_utils import run_bass_kernel_spmd
    nc = _get_nc(reps, loop=loop_iters is not None, ebufs=ebufs, pbufs=pbufs)
    if loop_iters is not None:
        it = np.array([[loop_iters]], dtype=np.int32)
        in_maps = [{**m, "iters": it} for m in in_maps]
    res = run_bass_kernel_spmd(nc, in_maps, core_ids=list(range(NCORES)))
    return np.concatenate([res.results[c]["out"] for c in range(NCORES)],
                          axis=0)


def kernel(x, weight_mu, weight_psi, bias_mu, bias_psi, eps_w, eps_b):
    in_maps = _prepare_inmaps(x, weight_mu, weight_psi, bias_mu, bias_psi,
                              eps_w, eps_b)
    try:
        return _run(in_maps)
    except Exception:
        _cache.clear()
        return _run(in_maps)


# revision 16
# speedup vs baseline: 1.1387x; 1.0055x over previous
"""Bayesian linear layer (mean-field reparameterization) on 8 TRN2 NeuronCores.

out[b,o] = sum_i (eps_w[b,o,i]*exp(w_psi[o,i]) + w_mu[o,i]) * x[b,i]
         + eps_b[b,o]*exp(b_psi[o]) + b_mu[o]

v7 strategy (data-parallel over batch, 32 batches/core):
 - eps_w shipped as fp8 e4m3 [i, b, o] (~2.7% rms elem noise on a term
   that is ~24% of the output -> ~6e-3 total rel err, gate is 2e-2).
 - The s=exp(psi) elementwise multiply must run on DVE (TT bf16 2x mode,
   4.42us per [128,8,1024] tile, measured); GPSIMD TT poisons DVE via
   the shared SBUF port (4.3x slowdown, measured) so all 32 tiles
   multiply on DVE: ~141us/rep, the kernel's wall.
 - To keep the DMA fabric under that wall, only 18/32 tiles use the
   SWDGE cast-DMA (fp8->bf16 doubles SBUF-side bytes); 14/32 are DMA'd
   raw fp8 (HWDGE) and upconverted on the idle scalar engine
   (7.1us/tile measured). DMA and ACT both stay under the DVE wall.
 - exp(psi) is computed into two half-sized s tiles so the next
   iteration's exp overlaps the previous iteration's tail multiplies
   (per-tile WAR tracking).
 - PE: per-batch matvecs col-tiled 4-wide (tile_position (0,32j),
   j=b%4): 4 M=32 matmuls stream concurrently (~2.6x measured).
   mu-term matmuls (x @ muT, bf16) and the b_mu row (K=1 ones matmul)
   accumulate into PSUM region 0. Epilogue: region sums + the
   eps_b*exp(b_psi) bias on DVE (one PSUM input per op).
 - params psi/mu/x/b_mu shipped bf16 (mu-term keeps bf16 accuracy).
"""

import numpy as np
import ml_dtypes

import os

BS, OUT, IN = 256, 1024, 1024
NCORES = 8
BPC = BS // NCORES          # 32 batches per core
ICH = IN // 128             # 8 i-chunks
EBUFS_DEFAULT = int(os.environ.get("BK_EBUFS", "7"))
E8BUFS_DEFAULT = int(os.environ.get("BK_E8BUFS", "3"))
ACT_TILES = int(os.environ.get("BK_NACT", "14"))  # raw+ACT-upconvert tiles
MU_CPT = 4                  # i-chunks per mu/psi DMA

_cache = {}


def _build(reps, loop=False, ebufs=None, pbufs=None):
    EBUFS = ebufs or EBUFS_DEFAULT
    import concourse.bass as bass
    import concourse.mybir as mybir
    import concourse.bacc as bacc
    from concourse import tile

    f32 = mybir.dt.float32
    bf16 = mybir.dt.bfloat16
    fp8 = mybir.dt.float8e4
    mult = mybir.AluOpType.mult
    add = mybir.AluOpType.add

    nc = bacc.Bacc(None, target_bir_lowering=False)

    d_eps = nc.dram_tensor("epsT", [IN, BPC, OUT], fp8, kind="ExternalInput")
    d_xT = nc.dram_tensor("xT", [IN, BPC], bf16, kind="ExternalInput")
    d_psiT = nc.dram_tensor("psiT", [IN, OUT], bf16, kind="ExternalInput")
    d_muT = nc.dram_tensor("muT", [IN, OUT], bf16, kind="ExternalInput")
    d_eb = nc.dram_tensor("eps_b", [BPC, OUT], f32, kind="ExternalInput")
    d_bpsi = nc.dram_tensor("bpsi", [1, OUT], f32, kind="ExternalInput")
    d_bmu = nc.dram_tensor("bmu16", [1, OUT], bf16, kind="ExternalInput")
    if loop:
        d_it = nc.dram_tensor("iters", [1, 1], mybir.dt.int32,
                              kind="ExternalInput")
    d_out = nc.dram_tensor("out", [BPC, OUT], f32, kind="ExternalOutput")

    with tile.TileContext(nc) as tc:
        with tc.tile_pool(name="const", bufs=1) as cpool, \
             tc.tile_pool(name="eps", bufs=EBUFS) as epool, \
             tc.tile_pool(name="eps8", bufs=E8BUFS_DEFAULT) as e8pool, \
             tc.tile_pool(name="ps", bufs=2, space="PSUM") as pspool:

            sTbA = cpool.tile([128, ICH // 2, OUT], bf16, name="sTbA")
            sTbB = cpool.tile([128, ICH // 2, OUT], bf16, name="sTbB")
            xTb = cpool.tile([128, ICH, BPC], bf16, name="xTb")
            xdiag = cpool.tile([128, ICH, BPC, BPC], bf16, name="xdiag")
            ones = cpool.tile([1, BPC], bf16, name="ones")
            ebt = cpool.tile([BPC, OUT], f32, name="ebt")
            sbrow = cpool.tile([1, OUT], f32, name="sbrow")
            sb_bc = cpool.tile([BPC, OUT], f32, name="sb_bc")
            bmurow = cpool.tile([1, OUT], bf16, name="bmurow")
            t1 = cpool.tile([BPC, OUT], f32, name="t1")
            t2 = cpool.tile([BPC, OUT], f32, name="t2")
            ebs = cpool.tile([BPC, OUT], f32, name="ebs")
            out_sb = cpool.tile([BPC, OUT], f32, name="out_sb")

            nc.vector.memset(xdiag[:], 0.0)
            nc.vector.memset(ones[:], 1.0)

            def emit(rep):
                # ---- prologue: params, exp(psi) -> bf16 ----
                for t in range(ICH // MU_CPT):
                    pt = epool.tile([128, MU_CPT, OUT], bf16,
                                    name=f"psi_{rep}_{t}", tag="eps")
                    nc.scalar.dma_start(
                        out=pt[:],
                        in_=d_psiT[t * MU_CPT * 128:(t + 1) * MU_CPT * 128, :]
                        .rearrange("(s p) o -> p s o", p=128))
                    nc.scalar.activation(
                        (sTbA if t == 0 else sTbB)[:], pt[:],
                        mybir.ActivationFunctionType.Exp)
                nc.scalar.dma_start(out=xTb[:], in_=d_xT[:]
                                    .rearrange("(c p) b -> p c b", p=128))
                # xdiag[:, :, b, b] = x[b, :] ; off-diagonal stays zero
                for b in range(BPC):
                    nc.scalar.copy(xdiag[:, :, b, b], xTb[:, :, b])

                nc.sync.dma_start(out=ebt[:], in_=d_eb[:])
                nc.sync.dma_start(out=sbrow[:], in_=d_bpsi[:])
                nc.scalar.activation(sbrow[:], sbrow[:],
                                     mybir.ActivationFunctionType.Exp)
                nc.gpsimd.partition_broadcast(sb_bc[:], sbrow[:])
                nc.sync.dma_start(out=bmurow[:], in_=d_bmu[:])

                # PSUM: 4 col-group regions of 32 rows; region j
                # accumulates batches with b%4==j (plus mu-term+b_mu in 0)
                accps = pspool.tile([128, OUT], f32, name=f"acc_{rep}",
                                    tag="ps")

                # mu-term into region 0: acc[b,o] += sum_i x[b,i]*mu[o,i]
                for t in range(ICH // MU_CPT):
                    mt = epool.tile([128, MU_CPT, OUT], bf16,
                                    name=f"mu_{rep}_{t}", tag="eps")
                    nc.scalar.dma_start(
                        out=mt[:],
                        in_=d_muT[t * MU_CPT * 128:(t + 1) * MU_CPT * 128, :]
                        .rearrange("(s p) o -> p s o", p=128))
                    for s in range(MU_CPT):
                        ic = t * MU_CPT + s
                        for h in range(2):
                            nc.tensor.matmul(
                                accps[0:BPC, h * 512:(h + 1) * 512],
                                xTb[:, ic, :],
                                mt[:, s, h * 512:(h + 1) * 512],
                                start=(ic == 0), stop=False)
                # b_mu row broadcast into all 32 rows of region 0 (K=1)
                for h in range(2):
                    nc.tensor.matmul(
                        accps[0:BPC, h * 512:(h + 1) * 512],
                        ones[:], bmurow[0:1, h * 512:(h + 1) * 512],
                        start=False, stop=False)

                # ---- main loop: eps-term matvecs, col-tiled 4-wide ----
                BG = 8                       # batches per eps tile
                NG = BPC // BG
                started = [True, False, False, False]
                # raw+ACT-upconvert assignment: g==1 all ics, g==2 even ics
                def is_act(ic, g):
                    k = ic * NG + g
                    return (g == 1 and ic < ACT_TILES) or \
                           (g == 2 and ic < ACT_TILES - 8)
                for ic in range(ICH):
                    for g in range(NG):
                        e = epool.tile([128, BG, OUT], bf16,
                                       name=f"e_{rep}_{ic}_{g}",
                                       tag="eps")
                        if is_act(ic, g):
                            # raw fp8 DMA (HWDGE) + scalar-engine upconvert
                            e8 = e8pool.tile([128, BG, OUT], fp8,
                                             name=f"e8_{rep}_{ic}_{g}",
                                             tag="e8")
                            nc.sync.dma_start(
                                out=e8[:],
                                in_=d_eps[ic * 128:(ic + 1) * 128,
                                          g * BG:(g + 1) * BG, :])
                            nc.scalar.copy(e[:], e8[:])
                        else:
                            # cast-DMA fp8->bf16 (SWDGE)
                            nc.gpsimd.dma_start(
                                out=e[:],
                                in_=d_eps[ic * 128:(ic + 1) * 128,
                                          g * BG:(g + 1) * BG, :])
                        # DVE multiply by s, in place
                        sT = sTbA if ic < ICH // 2 else sTbB
                        ics = ic % (ICH // 2)
                        nc.vector.tensor_tensor(
                            e[:], e[:],
                            sT[:, ics:ics + 1, :].broadcast_to(
                                (128, BG, OUT)), mult)
                        p2 = e
                        last = ic == ICH - 1
                        for bj in range(BG):
                            b = g * BG + bj
                            j = b % 4
                            st = not started[j]
                            started[j] = True
                            # last matmul for region j: ic==7, last bj in
                            # this tile with bj%4==j (bj = j+4), last g
                            sp = last and g == NG - 1 and bj == j + 4
                            for h in range(2):
                                nc.tensor.matmul(
                                    accps[32 * j:32 * (j + 1),
                                          h * 512:(h + 1) * 512],
                                    xdiag[:, ic, b, :],
                                    p2[:, bj, h * 512:(h + 1) * 512],
                                    start=st, stop=sp,
                                    tile_position=(0, 32 * j))

                # ---- epilogue: bias + region sums (1 PSUM input/op;
                # PSUM base may differ from the SBUF operand's base) ----
                nc.vector.tensor_tensor(ebs[:], ebt[:], sb_bc[:], mult)
                nc.vector.tensor_tensor(t1[:], accps[0:32, :], ebs[:], add)
                nc.vector.tensor_tensor(t2[:], accps[32:64, :], t1[:], add)
                nc.vector.tensor_tensor(t1[:], accps[64:96, :], t2[:], add)
                nc.vector.tensor_tensor(out_sb[:], accps[96:128, :], t1[:],
                                        add)
                nc.sync.dma_start(out=d_out[:], in_=out_sb[:])

            if loop:
                it_sb = cpool.tile([1, 1], mybir.dt.int32, name="it_sb")
                nc.sync.dma_start(out=it_sb[:], in_=d_it[:])
                regs = []
                for et in mybir.ALL_ENGINES:
                    eng = nc.engines[et]
                    r = eng.alloc_register(f"iters_{et.name}")
                    eng.reg_load(r, it_sb[0:1, 0:1])
                    regs.append(r)
                iters_val = bass.make_scalar_value(
                    bass.RegisterHandles(regs), min_val=1, max_val=1 << 20)
                with tc.For_i(0, iters_val, 1,
                              hint_engines=(mybir.EngineType.PE,
                                            mybir.EngineType.DVE,
                                            mybir.EngineType.SP)):
                    emit(0)
            else:
                for rep in range(reps):
                    emit(rep)

    nc.compile()
    return nc


def _get_nc(reps, loop=False, ebufs=None, pbufs=None):
    key = (reps, loop, ebufs, pbufs)
    if key not in _cache:
        _cache[key] = _build(reps, loop, ebufs, pbufs)
    return _cache[key]


def _prepare_inmaps(x, weight_mu, weight_psi, bias_mu, bias_psi, eps_w, eps_b):
    x = np.asarray(x, dtype=np.float32)
    weight_mu = np.asarray(weight_mu, dtype=np.float32)
    weight_psi = np.asarray(weight_psi, dtype=np.float32)
    bias_mu = np.asarray(bias_mu, dtype=np.float32)
    bias_psi = np.asarray(bias_psi, dtype=np.float32)
    eps_w = np.asarray(eps_w, dtype=np.float32)
    eps_b = np.asarray(eps_b, dtype=np.float32)

    # fp8 cast once on the full tensor (cheaper transposes afterwards).
    # TRN FP8_EXP4 tops out at +-240; N(0,1) data never gets near it.
    eps8 = eps_w.astype(ml_dtypes.float8_e4m3)
    psiT = np.ascontiguousarray(weight_psi.T).astype(ml_dtypes.bfloat16)
    muT = np.ascontiguousarray(weight_mu.T).astype(ml_dtypes.bfloat16)
    bpsi = bias_psi.reshape(1, OUT)
    bmu16 = bias_mu.reshape(1, OUT).astype(ml_dtypes.bfloat16)

    in_maps = []
    for c in range(NCORES):
        sl = slice(c * BPC, (c + 1) * BPC)
        in_maps.append({
            "epsT": np.ascontiguousarray(eps8[sl].transpose(2, 0, 1)),
            "xT": np.ascontiguousarray(x[sl].T).astype(ml_dtypes.bfloat16),
            "psiT": psiT,
            "muT": muT,
            "eps_b": np.ascontiguousarray(eps_b[sl]),
            "bpsi": bpsi,
            "bmu16": bmu16,
        })
    return in_maps


def _run(in_maps, reps=1, loop_iters=None, ebufs=None, pbufs=None, **kw):
    from concourse.bass_utils import run_bass_kernel_spmd
    nc = _get_nc(reps, loop=loop_iters is not None, ebufs=ebufs, pbufs=pbufs)
    if loop_iters is not None:
        it = np.array([[loop_iters]], dtype=np.int32)
        in_maps = [{**m, "iters": it} for m in in_maps]
    res = run_bass_kernel_spmd(nc, in_maps, core_ids=list(range(NCORES)))
    return np.concatenate([res.results[c]["out"] for c in range(NCORES)],
                          axis=0)


def kernel(x, weight_mu, weight_psi, bias_mu, bias_psi, eps_w, eps_b):
    in_maps = _prepare_inmaps(x, weight_mu, weight_psi, bias_mu, bias_psi,
                              eps_w, eps_b)
    try:
        return _run(in_maps)
    except Exception:
        _cache.clear()
        return _run(in_maps)


# revision 17
# speedup vs baseline: 1.1647x; 1.0228x over previous
"""Bayesian linear layer (mean-field reparameterization) on 8 TRN2 NeuronCores.

out[b,o] = sum_i (eps_w[b,o,i]*exp(w_psi[o,i]) + w_mu[o,i]) * x[b,i]
         + eps_b[b,o]*exp(b_psi[o]) + b_mu[o]

v7 strategy (data-parallel over batch, 32 batches/core):
 - eps_w shipped as fp8 e4m3 [i, b, o] (~2.7% rms elem noise on a term
   that is ~24% of the output -> ~6e-3 total rel err, gate is 2e-2).
 - The s=exp(psi) elementwise multiply must run on DVE (TT bf16 2x mode,
   4.42us per [128,8,1024] tile, measured); GPSIMD TT poisons DVE via
   the shared SBUF port (4.3x slowdown, measured) so all 32 tiles
   multiply on DVE: ~141us/rep, the kernel's wall.
 - To keep the DMA fabric under that wall, only 18/32 tiles use the
   SWDGE cast-DMA (fp8->bf16 doubles SBUF-side bytes); 14/32 are DMA'd
   raw fp8 (HWDGE) and upconverted on the idle scalar engine
   (7.1us/tile measured). DMA and ACT both stay under the DVE wall.
 - exp(psi) is computed into two half-sized s tiles so the next
   iteration's exp overlaps the previous iteration's tail multiplies
   (per-tile WAR tracking).
 - PE: per-batch matvecs col-tiled 4-wide (tile_position (0,32j),
   j=b%4): 4 M=32 matmuls stream concurrently (~2.6x measured).
   mu-term matmuls (x @ muT, bf16) and the b_mu row (K=1 ones matmul)
   accumulate into PSUM region 0. Epilogue: region sums + the
   eps_b*exp(b_psi) bias on DVE (one PSUM input per op).
 - params psi/mu/x/b_mu shipped bf16 (mu-term keeps bf16 accuracy).
"""

import numpy as np
import ml_dtypes

import os

BS, OUT, IN = 256, 1024, 1024
NCORES = 8
BPC = BS // NCORES          # 32 batches per core
ICH = IN // 128             # 8 i-chunks
EBUFS_DEFAULT = int(os.environ.get("BK_EBUFS", "7"))
E8BUFS_DEFAULT = int(os.environ.get("BK_E8BUFS", "3"))
ACT_TILES = int(os.environ.get("BK_NACT", "14"))  # raw+ACT-upconvert tiles
MU_CPT = 4                  # i-chunks per mu/psi DMA

_cache = {}


def _build(reps, loop=False, ebufs=None, pbufs=None):
    EBUFS = ebufs or EBUFS_DEFAULT
    import concourse.bass as bass
    import concourse.mybir as mybir
    import concourse.bacc as bacc
    from concourse import tile

    f32 = mybir.dt.float32
    bf16 = mybir.dt.bfloat16
    fp8 = mybir.dt.float8e4
    mult = mybir.AluOpType.mult
    add = mybir.AluOpType.add

    nc = bacc.Bacc(None, target_bir_lowering=False)

    d_eps = nc.dram_tensor("epsT", [IN, BPC, OUT], fp8, kind="ExternalInput")
    d_xT = nc.dram_tensor("xT", [IN, BPC], bf16, kind="ExternalInput")
    d_psiT = nc.dram_tensor("psiT", [IN, OUT], bf16, kind="ExternalInput")
    d_muT = nc.dram_tensor("muT", [IN, OUT], bf16, kind="ExternalInput")
    d_eb = nc.dram_tensor("eps_b", [BPC, OUT], f32, kind="ExternalInput")
    d_bpsi = nc.dram_tensor("bpsi", [1, OUT], f32, kind="ExternalInput")
    d_bmu = nc.dram_tensor("bmu16", [1, OUT], bf16, kind="ExternalInput")
    if loop:
        d_it = nc.dram_tensor("iters", [1, 1], mybir.dt.int32,
                              kind="ExternalInput")
    d_out = nc.dram_tensor("out", [BPC, OUT], f32, kind="ExternalOutput")

    with tile.TileContext(nc) as tc:
        with tc.tile_pool(name="const", bufs=1) as cpool, \
             tc.tile_pool(name="eps", bufs=EBUFS) as epool, \
             tc.tile_pool(name="eps8", bufs=E8BUFS_DEFAULT) as e8pool, \
             tc.tile_pool(name="ps", bufs=4, space="PSUM") as pspool:

            sTbA = cpool.tile([128, ICH // 2, OUT], bf16, name="sTbA")
            sTbB = cpool.tile([128, ICH // 2, OUT], bf16, name="sTbB")
            xTb = cpool.tile([128, ICH, BPC], bf16, name="xTb")
            xdiag = cpool.tile([128, ICH, BPC, BPC], bf16, name="xdiag")
            ones = cpool.tile([1, BPC], bf16, name="ones")
            ebt = cpool.tile([BPC, OUT], f32, name="ebt")
            sbrow = cpool.tile([1, OUT], f32, name="sbrow")
            sb_bc = cpool.tile([BPC, OUT], f32, name="sb_bc")
            bmurow = cpool.tile([1, OUT], bf16, name="bmurow")
            t1 = cpool.tile([BPC, OUT], f32, name="t1")
            t2 = cpool.tile([BPC, OUT], f32, name="t2")
            ebs = cpool.tile([BPC, OUT], f32, name="ebs")
            out_sb = cpool.tile([BPC, OUT], f32, name="out_sb")

            nc.vector.memset(xdiag[:], 0.0)
            nc.vector.memset(ones[:], 1.0)

            def emit(rep):
                # ---- prologue: params, exp(psi) -> bf16 ----
                for t in range(ICH // MU_CPT):
                    pt = epool.tile([128, MU_CPT, OUT], bf16,
                                    name=f"psi_{rep}_{t}", tag="eps")
                    nc.scalar.dma_start(
                        out=pt[:],
                        in_=d_psiT[t * MU_CPT * 128:(t + 1) * MU_CPT * 128, :]
                        .rearrange("(s p) o -> p s o", p=128))
                    nc.scalar.activation(
                        (sTbA if t == 0 else sTbB)[:], pt[:],
                        mybir.ActivationFunctionType.Exp)
                nc.scalar.dma_start(out=xTb[:], in_=d_xT[:]
                                    .rearrange("(c p) b -> p c b", p=128))
                # xdiag[:, :, b, b] = x[b, :] ; off-diagonal stays zero
                for b in range(BPC):
                    nc.scalar.copy(xdiag[:, :, b, b], xTb[:, :, b])

                nc.sync.dma_start(out=ebt[:], in_=d_eb[:])
                nc.sync.dma_start(out=sbrow[:], in_=d_bpsi[:])
                nc.scalar.activation(sbrow[:], sbrow[:],
                                     mybir.ActivationFunctionType.Exp)
                nc.gpsimd.partition_broadcast(sb_bc[:], sbrow[:])
                nc.vector.tensor_tensor(ebs[:], ebt[:], sb_bc[:], mult)
                nc.sync.dma_start(out=bmurow[:], in_=d_bmu[:])

                # PSUM: 4 col-group regions of 32 rows; region j
                # accumulates batches with b%4==j (plus mu-term+b_mu in 0)
                accps = pspool.tile([128, OUT], f32, name=f"acc_{rep}",
                                    tag="ps")

                # mu-term into region 0: acc[b,o] += sum_i x[b,i]*mu[o,i]
                for t in range(ICH // MU_CPT):
                    mt = epool.tile([128, MU_CPT, OUT], bf16,
                                    name=f"mu_{rep}_{t}", tag="eps")
                    nc.scalar.dma_start(
                        out=mt[:],
                        in_=d_muT[t * MU_CPT * 128:(t + 1) * MU_CPT * 128, :]
                        .rearrange("(s p) o -> p s o", p=128))
                    for s in range(MU_CPT):
                        ic = t * MU_CPT + s
                        for h in range(2):
                            nc.tensor.matmul(
                                accps[0:BPC, h * 512:(h + 1) * 512],
                                xTb[:, ic, :],
                                mt[:, s, h * 512:(h + 1) * 512],
                                start=(ic == 0), stop=False)
                # b_mu row broadcast into all 32 rows of region 0 (K=1)
                for h in range(2):
                    nc.tensor.matmul(
                        accps[0:BPC, h * 512:(h + 1) * 512],
                        ones[:], bmurow[0:1, h * 512:(h + 1) * 512],
                        start=False, stop=False)

                # ---- main loop: eps-term matvecs, col-tiled 4-wide ----
                BG = 8                       # batches per eps tile
                NG = BPC // BG
                started = [True, False, False, False]
                # raw+ACT-upconvert assignment: g==1 all ics, g==2 even ics
                def is_act(ic, g):
                    return (g == 1 and ic < ACT_TILES) or \
                           (g == 3 and ic < ACT_TILES - 8)
                for ic in range(ICH):
                    for g in range(NG):
                        e = epool.tile([128, BG, OUT], bf16,
                                       name=f"e_{rep}_{ic}_{g}",
                                       tag="eps")
                        if is_act(ic, g):
                            # raw fp8 DMA (HWDGE) + scalar-engine upconvert
                            e8 = e8pool.tile([128, BG, OUT], fp8,
                                             name=f"e8_{rep}_{ic}_{g}",
                                             tag="e8")
                            nc.sync.dma_start(
                                out=e8[:],
                                in_=d_eps[ic * 128:(ic + 1) * 128,
                                          g * BG:(g + 1) * BG, :])
                            nc.scalar.copy(e[:], e8[:])
                        else:
                            # cast-DMA fp8->bf16 (SWDGE)
                            nc.gpsimd.dma_start(
                                out=e[:],
                                in_=d_eps[ic * 128:(ic + 1) * 128,
                                          g * BG:(g + 1) * BG, :])
                        # DVE multiply by s, in place
                        sT = sTbA if ic < ICH // 2 else sTbB
                        ics = ic % (ICH // 2)
                        nc.vector.tensor_tensor(
                            e[:], e[:],
                            sT[:, ics:ics + 1, :].broadcast_to(
                                (128, BG, OUT)), mult)
                        p2 = e
                        last = ic == ICH - 1
                        for bj in range(BG):
                            b = g * BG + bj
                            j = b % 4
                            st = not started[j]
                            started[j] = True
                            # last matmul for region j: ic==7, last bj in
                            # this tile with bj%4==j (bj = j+4), last g
                            sp = last and g == NG - 1 and bj == j + 4
                            for h in range(2):
                                nc.tensor.matmul(
                                    accps[32 * j:32 * (j + 1),
                                          h * 512:(h + 1) * 512],
                                    xdiag[:, ic, b, :],
                                    p2[:, bj, h * 512:(h + 1) * 512],
                                    start=st, stop=sp,
                                    tile_position=(0, 32 * j))

                # ---- epilogue: bias + region sums (1 PSUM input/op;
                # PSUM base may differ from the SBUF operand's base) ----
                nc.vector.tensor_tensor(t1[:], accps[0:32, :], ebs[:], add)
                nc.vector.tensor_tensor(t2[:], accps[32:64, :], t1[:], add)
                nc.vector.tensor_tensor(t1[:], accps[64:96, :], t2[:], add)
                nc.vector.tensor_tensor(out_sb[:], accps[96:128, :], t1[:],
                                        add)
                nc.sync.dma_start(out=d_out[:], in_=out_sb[:])

            if loop:
                it_sb = cpool.tile([1, 1], mybir.dt.int32, name="it_sb")
                nc.sync.dma_start(out=it_sb[:], in_=d_it[:])
                regs = []
                for et in mybir.ALL_ENGINES:
                    eng = nc.engines[et]
                    r = eng.alloc_register(f"iters_{et.name}")
                    eng.reg_load(r, it_sb[0:1, 0:1])
                    regs.append(r)
                iters_val = bass.make_scalar_value(
                    bass.RegisterHandles(regs), min_val=1, max_val=1 << 20)
                with tc.For_i(0, iters_val, 1,
                              hint_engines=(mybir.EngineType.PE,
                                            mybir.EngineType.DVE,
                                            mybir.EngineType.SP)):
                    emit(0)
            else:
                for rep in range(reps):
                    emit(rep)

    nc.compile()
    return nc


def _get_nc(reps, loop=False, ebufs=None, pbufs=None):
    key = (reps, loop, ebufs, pbufs)
    if key not in _cache:
        _cache[key] = _build(reps, loop, ebufs, pbufs)
    return _cache[key]


def _prepare_inmaps(x, weight_mu, weight_psi, bias_mu, bias_psi, eps_w, eps_b):
    x = np.asarray(x, dtype=np.float32)
    weight_mu = np.asarray(weight_mu, dtype=np.float32)
    weight_psi = np.asarray(weight_psi, dtype=np.float32)
    bias_mu = np.asarray(bias_mu, dtype=np.float32)
    bias_psi = np.asarray(bias_psi, dtype=np.float32)
    eps_w = np.asarray(eps_w, dtype=np.float32)
    eps_b = np.asarray(eps_b, dtype=np.float32)

    # fp8 cast once on the full tensor (cheaper transposes afterwards).
    # TRN FP8_EXP4 tops out at +-240; N(0,1) data never gets near it.
    eps8 = eps_w.astype(ml_dtypes.float8_e4m3)
    psiT = np.ascontiguousarray(weight_psi.T).astype(ml_dtypes.bfloat16)
    muT = np.ascontiguousarray(weight_mu.T).astype(ml_dtypes.bfloat16)
    bpsi = bias_psi.reshape(1, OUT)
    bmu16 = bias_mu.reshape(1, OUT).astype(ml_dtypes.bfloat16)

    in_maps = []
    for c in range(NCORES):
        sl = slice(c * BPC, (c + 1) * BPC)
        in_maps.append({
            "epsT": np.ascontiguousarray(eps8[sl].transpose(2, 0, 1)),
            "xT": np.ascontiguousarray(x[sl].T).astype(ml_dtypes.bfloat16),
            "psiT": psiT,
            "muT": muT,
            "eps_b": np.ascontiguousarray(eps_b[sl]),
            "bpsi": bpsi,
            "bmu16": bmu16,
        })
    return in_maps


def _run(in_maps, reps=1, loop_iters=None, ebufs=None, pbufs=None, **kw):
    from concourse.bass_utils import run_bass_kernel_spmd
    nc = _get_nc(reps, loop=loop_iters is not None, ebufs=ebufs, pbufs=pbufs)
    if loop_iters is not None:
        it = np.array([[loop_iters]], dtype=np.int32)
        in_maps = [{**m, "iters": it} for m in in_maps]
    res = run_bass_kernel_spmd(nc, in_maps, core_ids=list(range(NCORES)))
    return np.concatenate([res.results[c]["out"] for c in range(NCORES)],
                          axis=0)


def kernel(x, weight_mu, weight_psi, bias_mu, bias_psi, eps_w, eps_b):
    in_maps = _prepare_inmaps(x, weight_mu, weight_psi, bias_mu, bias_psi,
                              eps_w, eps_b)
    try:
        return _run(in_maps)
    except Exception:
        _cache.clear()
        return _run(in_maps)
